# revision 9
# baseline (speedup 1.0000x reference)
"""Trainium2 Bass kernel for BodyStructureLoss (deinterleaved thirds).

Host deinterleaves each core's shard into [P, 3, 8704]: per partition row
[all x | all y | all z]. Per tile of M norm-columns:
  - one DMA brings [P, 3, M] (three contiguous M-runs per partition)
  - squares run in parallel per third (x->ACT, y->DVE, z->Pool mid-stream;
    the DVE-heavy z moves around in the tail), all fp32 -> bf16
  - s = x2+y2 (+z2) via two packed-bf16 DVE adds (2x mode)
  - count-TS (s>1, DVE 4x) accumulates per tile into its own acc column
Tiles are grouped in PAIRS sharing one s2 buffer; per pair one ACT sqrt
(bf16) + one max-TS (sum(max(d,1)), DVE 4x). Pairing gives the ACT queue
slack against the s->sqrt feedback loop; nosync scheduler edges keep each
sqrt BEHIND the newest tile's ACT square so squares stay DMA-anchored.
The final pair instead runs m=max(s,1) (DVE 4x) then an ACT
sqrt-with-accumulate, ending the critical chain on ACT with no post-sqrt
DVE hop. TS scratch outputs rotate per-op (a shared scratch would WAW-
serialize every accumulate against every other).
Host: sum acc, subtract P*8704 per core (count+max identity:
sum(max(d,1)) + count(s>1) = masked_sum + N), divide by B*J.
"""

import os

import numpy as np

os.environ["BASS_NEVER_TRACE"] = "1"

import concourse.bacc as bacc
import concourse.mybir as mybir
from concourse.bass_utils import run_bass_kernel_spmd
from concourse.tile import TileContext

B, J, D = 524288, 17, 3
N_CORES = 8
P = 128
M_TOT = B // N_CORES * J // P  # 8704 norm columns per partition
_DT = mybir.dt.float32
_BF = mybir.dt.bfloat16

# pairs of tile sizes (norm-columns); a 1-element pair gets its own sqrt
PLAN = [[160, 160]] + [[576, 576]] * 6 + [[336, 336], [176, 176],
                                          [160, 160], [128]]
assert sum(m for pr in PLAN for m in pr) == M_TOT


def build_nc(plan=None, lag=1, xbufs=4, sq_eng=("act", "dve", "pool"),
             tail_dve_sq=0, last_flush=True, tail_pool=0, tail_edge=3,
             tail_eng=("act", "pool", "dve"), tail_eng_n=7,
             last2_eng=("act", "pool", "pool"), tail_lag_extra=0,
             sqrt_acc_last=1, pin_max_tail=0, endgame_edges=True):
    import bass_rust
    from collections import Counter, deque
    NameSet = bass_rust.InstructionNameOrderedSet

    if plan is None:
        plan = PLAN
    flat = [m for pr in plan for m in pr]
    assert sum(flat) == M_TOT
    n_tiles = len(flat)
    n_pairs = len(plan)
    m_count = Counter(flat)
    m2_count = Counter(sum(pr) for pr in plan)

    nc = bacc.Bacc(
        "TRN2", target_bir_lowering=False, debug=False, num_devices=N_CORES
    )
    x = nc.dram_tensor("x", [P, 3, M_TOT], _DT, kind="ExternalInput")
    # acc columns: one count col per tile + one max col per pair
    acc_cols = n_tiles + n_pairs
    out = nc.dram_tensor("out", [P, acc_cols], _DT, kind="ExternalOutput")

    with TileContext(nc) as tc:
        with (
            tc.tile_pool(name="xin", bufs=xbufs) as xpool,
            tc.tile_pool(name="small", bufs=4) as spool,
            tc.tile_pool(name="accp", bufs=1) as accpool,
        ):
            eng_of = {"dve": nc.vector, "pool": nc.gpsimd}
            accs = accpool.tile([P, acc_cols], _DT)
            scr16 = accpool.tile([P, 2], _BF)  # dummy-sqrt operand only

            # dummy sqrt: preload the ACT table set containing Sqrt+Square
            nc.vector.memset(scr16[:, :1], 1.0)
            nc.scalar.activation(
                out=scr16[:, :1], in_=scr16[:, :1],
                func=mybir.ActivationFunctionType.Sqrt,
            )

            acc_writers = []
            wb_idx = accpool.tile([P, 1], mybir.dt.int32)
            nc.gpsimd.memset(wb_idx, 0)
            wb_sem = nc.alloc_semaphore("wb_dma")
            # scratch accum target: walrus rejects tensor_scalar without an
            # accum_out, so the elementwise max accumulates here (unused)
            dummy_acc = accpool.tile([P, 1], _DT)

            gi = [0]  # global tile index
            m_off = [0]
            sq_act_of = {}  # tile -> its ACT square handle (or None)
            last_max = [None]  # most recent max/tsmax handle (DVE)
            count_of = {}  # tile -> count handle
            tsmax_of = {}  # pair -> tsmax handle (sqrt_acc path)
            max_of = {}  # pair -> max handle (regular path)

            def stage_a(ti, M, s2, s_off):
                sz = str(M)
                nb = min(xbufs, m_count[M])
                xt = xpool.tile([P, 3, M], _DT, tag="xt" + sz, bufs=nb)
                m0 = m_off[0]
                r = nc.sync.dma_start(out=xt, in_=x[:, :, m0 : m0 + M])
                NAME_MAP[r.ins.name] = ("dma", ti)
                m_off[0] += M
                x2 = spool.tile([P, 3, M], _BF, tag="x2" + sz,
                                bufs=min(4, m_count[M]))
                ndve = tail_dve_sq and ti >= n_tiles - tail_dve_sq
                npool = tail_pool and ti >= n_tiles - tail_pool
                engs = sq_eng
                if tail_eng and ti >= n_tiles - tail_eng_n:
                    engs = tail_eng
                if last2_eng and ti >= n_tiles - 2:
                    engs = last2_eng
                sq_act = None
                for k, ename in enumerate(engs):
                    if ndve:
                        ename = "dve"
                    elif npool and k > 0:
                        # tail: y,z squares on Pool to unload DVE's queue
                        ename = "pool"
                    if ename == "act":
                        r = nc.scalar.activation(
                            out=x2[:, k, :], in_=xt[:, k, :],
                            func=mybir.ActivationFunctionType.Square,
                        )
                        sq_act = r
                    else:
                        r = eng_of[ename].tensor_tensor(
                            out=x2[:, k, :], in0=xt[:, k, :], in1=xt[:, k, :],
                            op=mybir.AluOpType.mult,
                        )
                    NAME_MAP[r.ins.name] = ("sq_" + "xyz"[k], ti)
                sq_act_of[ti] = sq_act
                sl = s2[:, s_off : s_off + M]
                r = nc.vector.tensor_tensor(
                    out=sl, in0=x2[:, 0, :], in1=x2[:, 1, :],
                    op=mybir.AluOpType.add,
                )
                NAME_MAP[r.ins.name] = ("add1", ti)
                if (pin_max_tail and ti >= n_tiles - pin_max_tail
                        and last_max[0] is not None):
                    # keep earlier pairs' max ops AHEAD of the tail adds on
                    # DVE so they don't pollute the endgame queue
                    r.ins.set_nosync_dependencies(NameSet(
                        list(r.ins.nosync_dependency_names())
                        + [last_max[0].ins.name]
                    ))
                r = nc.vector.tensor_tensor(
                    out=sl, in0=sl, in1=x2[:, 2, :], op=mybir.AluOpType.add,
                )
                NAME_MAP[r.ins.name] = ("add2", ti)
                # count(s > 1) -> acc col ti (4x TS; independent of sqrt)
                cscr = spool.tile([P, M], _BF, tag="c" + sz,
                                  bufs=min(2, m_count[M]))
                r = nc.vector.tensor_scalar(
                    out=cscr, in0=sl, scalar1=1.0, scalar2=None,
                    op0=mybir.AluOpType.is_gt, op1=mybir.AluOpType.add,
                    accum_out=accs[:, ti : ti + 1],
                )
                NAME_MAP[r.ins.name] = ("count", ti)
                count_of[ti] = r
                acc_writers.append(r)

            def stage_b(pi, M2, s2, after=None):
                sz = str(M2)
                use_sqrt_acc = sqrt_acc_last and pi >= n_pairs - sqrt_acc_last
                d = spool.tile([P, M2], _BF, tag="d" + sz,
                               bufs=min(4, m2_count[M2]))
                if use_sqrt_acc:
                    # m = max(s,1) on DVE (4x, right after add2 in-queue),
                    # then ACT sqrt-with-accum: acc += sum(sqrt(m)) =
                    # sum(max(d,1)). Ends on ACT -> no post-sqrt DVE hop.
                    m = spool.tile([P, M2], _BF, tag="m" + sz,
                                   bufs=min(2, m2_count[M2]))
                    r = nc.vector.tensor_scalar(
                        out=m, in0=s2, scalar1=1.0, scalar2=None,
                        op0=mybir.AluOpType.max, op1=mybir.AluOpType.add,
                        accum_out=dummy_acc,
                    )
                    NAME_MAP[r.ins.name] = ("tsmax", pi)
                    tsmax_of[pi] = r
                    r = nc.scalar.activation(
                        out=d, in_=m, func=mybir.ActivationFunctionType.Sqrt,
                        accum_out=accs[:, n_tiles + pi : n_tiles + pi + 1],
                    )
                    NAME_MAP[r.ins.name] = ("sqrt", pi)
                    if after is not None:
                        r.ins.set_nosync_dependencies(NameSet(
                            list(r.ins.nosync_dependency_names())
                            + [after.ins.name]
                        ))
                    acc_writers.append(r)
                    return
                r = nc.scalar.activation(
                    out=d, in_=s2, func=mybir.ActivationFunctionType.Sqrt,
                )
                NAME_MAP[r.ins.name] = ("sqrt", pi)
                if after is not None:
                    # scheduler-only edge: keep this sqrt BEHIND the newest
                    # tile's ACT square so squares stay DMA-anchored
                    r.ins.set_nosync_dependencies(NameSet(
                        list(r.ins.nosync_dependency_names())
                        + [after.ins.name]
                    ))
                # sum(max(d,1)) -> acc col n_tiles+pi (4x TS)
                mscr = spool.tile([P, M2], _BF, tag="mx" + sz,
                                  bufs=min(2, m2_count[M2]))
                r = nc.vector.tensor_scalar(
                    out=mscr, in0=d, scalar1=1.0, scalar2=None,
                    op0=mybir.AluOpType.max, op1=mybir.AluOpType.add,
                    accum_out=accs[:, n_tiles + pi : n_tiles + pi + 1],
                )
                NAME_MAP[r.ins.name] = ("max", pi)
                max_of[pi] = r
                acc_writers.append(r)
                last_max[0] = r

            pending = deque()
            for pi, pr in enumerate(plan):
                M2 = sum(pr)
                s2 = spool.tile([P, M2], _BF, tag="s" + str(M2),
                                bufs=min(4, m2_count[M2]))
                s_off = 0
                for M in pr:
                    stage_a(gi[0], M, s2, s_off)
                    s_off += M
                    gi[0] += 1
                pending.append((pi, M2, s2))
                last_pair = pi == n_pairs - 1
                eff_lag = lag
                if pi >= n_pairs - tail_edge:
                    eff_lag = lag + tail_lag_extra
                if last_flush and last_pair:
                    eff_lag = 0
                while len(pending) > eff_lag:
                    if pi >= n_pairs - tail_edge:
                        # tail: order sqrts after the newest ACT square
                        aft = None
                        for tj in range(gi[0] - 1, -1, -1):
                            if sq_act_of.get(tj) is not None:
                                aft = sq_act_of[tj]
                                break
                    else:
                        aft = sq_act_of.get(gi[0] - len(pr))
                    stage_b(*pending.popleft(), after=aft)
            while pending:
                stage_b(*pending.popleft())

            if endgame_edges:
                def add_nosync(ins_r, after_r):
                    nm = after_r.ins.name
                    if (nm in ins_r.ins.sync_dependency_names()
                            or nm in ins_r.ins.nosync_dependency_names()):
                        return
                    ins_r.ins.set_nosync_dependencies(NameSet(
                        list(ins_r.ins.nosync_dependency_names()) + [nm]
                    ))
                # keep late mid-pair maxes out of the endgame DVE chain:
                # order them after the last tile's count
                lc = count_of.get(n_tiles - 1)
                if lc is not None:
                    for pj in range(max(0, n_pairs - 4), n_pairs):
                        if pj in max_of:
                            add_nosync(max_of[pj], lc)
                # let the last count overlap the final sqrt: order it after
                # the last pair's tsmax on DVE
                tm = tsmax_of.get(n_pairs - 1)
                if tm is not None and lc is not None:
                    add_nosync(lc, tm)

            # out-DMA via pre-staged SWDGE descriptors (prep early, trigger
            # after the final accumulate; RAW edges moved to the trigger)
            in_view = accs[:, :].rearrange("p (a b w) -> p a b w", a=1, b=1)
            out_view = out[:, :].rearrange("p (a b w) -> a p b w", a=1, b=1)
            wb_prep = nc.gpsimd.kv_writeback(
                out_view, in_view, wb_idx[:, :], prepare_only=True, sem=wb_sem,
            )
            wb_trig = nc.gpsimd.trigger_dma(count=None)
            acc_names = {w.ins.name for w in acc_writers}
            prep_sync = list(wb_prep.ins.sync_dependency_names())
            wb_prep.ins.set_sync_dependencies(
                NameSet([d for d in prep_sync if d not in acc_names])
            )
            wb_prep.ins.set_nosync_dependencies(NameSet(
                [d for d in wb_prep.ins.nosync_dependency_names()
                 if d not in acc_names]
            ))
            wb_trig.ins.set_sync_dependencies(NameSet(
                list(wb_trig.ins.sync_dependency_names()) + sorted(acc_names)
            ))

    nc.compile()

    # point the prep's on_update[0] at the DMASW drain sem (scatter_add-style
    # wiring; see v1 kernel for rationale)
    dmasw = None
    for i in nc.all_instructions():
        if i.sync_info:
            for w in i.sync_info.on_wait:
                if w.ant_name and w.ant_name.startswith("DMASW"):
                    dmasw = (w.id, w.ant_name)
    assert dmasw is not None, "no DMASW drain wait found"
    wb_prep.ins.sync_info.on_update[0] = mybir.SyncUpdate(
        sync_type="semaphore", id=dmasw[0], ant_name=dmasw[1],
        update_mode="sem-add-imm", update_value=16,
    )
    return nc


NAME_MAP = {}  # ins name -> (kind, index) for trace attribution

_nc_cache = None
last_results = None


def kernel(kps_world_pred: np.ndarray) -> np.ndarray:
    global _nc_cache, last_results
    x = np.ascontiguousarray(kps_world_pred, dtype=np.float32)
    assert x.shape == (B, J, D)

    # shard + deinterleave: [8, P, 8704 triplets, 3] -> [8, P, 3, 8704]
    v = np.ascontiguousarray(
        x.reshape(N_CORES, P, M_TOT, 3).transpose(0, 1, 3, 2)
    )
    in_maps = [{"x": v[c]} for c in range(N_CORES)]

    if _nc_cache is None:
        _nc_cache = build_nc()

    import time

    res = None
    for attempt in range(3):
        try:
            res = run_bass_kernel_spmd(_nc_cache, in_maps, list(range(N_CORES)))
            break
        except Exception:
            if attempt == 2:
                raise
            time.sleep(15)
    last_results = res

    # identity: sum(max(d,1)) + count(s>1) = masked_sum + P*M_TOT per core
    total = np.float64(0.0)
    for c in range(N_CORES):
        total += res.results[c]["out"].astype(np.float64).sum()
    total -= np.float64(N_CORES * P * M_TOT)
    return np.asarray(total / (B * J), dtype=np.float32)


# revision 10
# speedup vs baseline: 1.0029x; 1.0029x over previous
"""Trainium2 Bass kernel for BodyStructureLoss (deinterleaved thirds).

Host deinterleaves each core's shard into [P, 3, 8704]: per partition row
[all x | all y | all z]. Per tile of M norm-columns:
  - one DMA brings [P, 3, M] (three contiguous M-runs per partition)
  - squares run in parallel per third (x->ACT, y->DVE, z->Pool mid-stream;
    the DVE-heavy z moves around in the tail), all fp32 -> bf16
  - s = x2+y2 (+z2) via two packed-bf16 DVE adds (2x mode)
  - count-TS (s>1, DVE 4x) accumulates per tile into its own acc column
Tiles are grouped in PAIRS sharing one s2 buffer; per pair one ACT sqrt
(bf16) + one max-TS (sum(max(d,1)), DVE 4x). Pairing gives the ACT queue
slack against the s->sqrt feedback loop; nosync scheduler edges keep each
sqrt BEHIND the newest tile's ACT square so squares stay DMA-anchored.
The final pair instead runs m=max(s,1) (DVE 4x) then an ACT
sqrt-with-accumulate, ending the critical chain on ACT with no post-sqrt
DVE hop. TS scratch outputs rotate per-op (a shared scratch would WAW-
serialize every accumulate against every other).
Host: sum acc, subtract P*8704 per core (count+max identity:
sum(max(d,1)) + count(s>1) = masked_sum + N), divide by B*J.
"""

import os

import numpy as np

os.environ["BASS_NEVER_TRACE"] = "1"

import concourse.bacc as bacc
import concourse.mybir as mybir
from concourse.bass_utils import run_bass_kernel_spmd
from concourse.tile import TileContext

B, J, D = 524288, 17, 3
N_CORES = 8
P = 128
M_TOT = B // N_CORES * J // P  # 8704 norm columns per partition
_DT = mybir.dt.float32
_BF = mybir.dt.bfloat16

# pairs of tile sizes (norm-columns); a 1-element pair gets its own sqrt
PLAN = [[160, 160]] + [[576, 576]] * 6 + [[336], [336], [176, 176],
                                          [160, 160], [128]]
assert sum(m for pr in PLAN for m in pr) == M_TOT


def build_nc(plan=None, lag=1, xbufs=4, sq_eng=("act", "dve", "pool"),
             tail_dve_sq=0, last_flush=True, tail_pool=0, tail_edge=3,
             tail_eng=("act", "pool", "dve"), tail_eng_n=7,
             last2_eng=("act", "pool", "pool"), tail_lag_extra=0,
             sqrt_acc_last=1, pin_max_tail=0, endgame_edges=True,
             eg_ct=2, eg_sq=2):
    import bass_rust
    from collections import Counter, deque
    NameSet = bass_rust.InstructionNameOrderedSet

    if plan is None:
        plan = PLAN
    flat = [m for pr in plan for m in pr]
    assert sum(flat) == M_TOT
    n_tiles = len(flat)
    n_pairs = len(plan)
    m_count = Counter(flat)
    m2_count = Counter(sum(pr) for pr in plan)

    nc = bacc.Bacc(
        "TRN2", target_bir_lowering=False, debug=False, num_devices=N_CORES
    )
    x = nc.dram_tensor("x", [P, 3, M_TOT], _DT, kind="ExternalInput")
    # acc columns: one count col per tile + one max col per pair
    acc_cols = n_tiles + n_pairs
    out = nc.dram_tensor("out", [P, acc_cols], _DT, kind="ExternalOutput")

    with TileContext(nc) as tc:
        with (
            tc.tile_pool(name="xin", bufs=xbufs) as xpool,
            tc.tile_pool(name="small", bufs=4) as spool,
            tc.tile_pool(name="accp", bufs=1) as accpool,
        ):
            eng_of = {"dve": nc.vector, "pool": nc.gpsimd}
            accs = accpool.tile([P, acc_cols], _DT)
            scr16 = accpool.tile([P, 2], _BF)  # dummy-sqrt operand only

            # dummy sqrt: preload the ACT table set containing Sqrt+Square
            nc.vector.memset(scr16[:, :1], 1.0)
            nc.scalar.activation(
                out=scr16[:, :1], in_=scr16[:, :1],
                func=mybir.ActivationFunctionType.Sqrt,
            )

            acc_writers = []
            wb_idx = accpool.tile([P, 1], mybir.dt.int32)
            nc.gpsimd.memset(wb_idx, 0)
            wb_sem = nc.alloc_semaphore("wb_dma")
            # scratch accum target: walrus rejects tensor_scalar without an
            # accum_out, so the elementwise max accumulates here (unused)
            dummy_acc = accpool.tile([P, 1], _DT)

            gi = [0]  # global tile index
            m_off = [0]
            sq_act_of = {}  # tile -> its ACT square handle (or None)
            last_max = [None]  # most recent max/tsmax handle (DVE)
            count_of = {}  # tile -> count handle
            tsmax_of = {}  # pair -> tsmax handle (sqrt_acc path)
            max_of = {}  # pair -> max handle (regular path)
            sqrt_of = {}  # pair -> sqrt handle

            def stage_a(ti, M, s2, s_off):
                sz = str(M)
                nb = min(xbufs, m_count[M])
                xt = xpool.tile([P, 3, M], _DT, tag="xt" + sz, bufs=nb)
                m0 = m_off[0]
                r = nc.sync.dma_start(out=xt, in_=x[:, :, m0 : m0 + M])
                NAME_MAP[r.ins.name] = ("dma", ti)
                m_off[0] += M
                x2 = spool.tile([P, 3, M], _BF, tag="x2" + sz,
                                bufs=min(4, m_count[M]))
                ndve = tail_dve_sq and ti >= n_tiles - tail_dve_sq
                npool = tail_pool and ti >= n_tiles - tail_pool
                engs = sq_eng
                if tail_eng and ti >= n_tiles - tail_eng_n:
                    engs = tail_eng
                if last2_eng and ti >= n_tiles - 2:
                    engs = last2_eng
                sq_act = None
                for k, ename in enumerate(engs):
                    if ndve:
                        ename = "dve"
                    elif npool and k > 0:
                        # tail: y,z squares on Pool to unload DVE's queue
                        ename = "pool"
                    if ename == "act":
                        r = nc.scalar.activation(
                            out=x2[:, k, :], in_=xt[:, k, :],
                            func=mybir.ActivationFunctionType.Square,
                        )
                        sq_act = r
                    else:
                        r = eng_of[ename].tensor_tensor(
                            out=x2[:, k, :], in0=xt[:, k, :], in1=xt[:, k, :],
                            op=mybir.AluOpType.mult,
                        )
                    NAME_MAP[r.ins.name] = ("sq_" + "xyz"[k], ti)
                sq_act_of[ti] = sq_act
                sl = s2[:, s_off : s_off + M]
                r = nc.vector.tensor_tensor(
                    out=sl, in0=x2[:, 0, :], in1=x2[:, 1, :],
                    op=mybir.AluOpType.add,
                )
                NAME_MAP[r.ins.name] = ("add1", ti)
                if (pin_max_tail and ti >= n_tiles - pin_max_tail
                        and last_max[0] is not None):
                    # keep earlier pairs' max ops AHEAD of the tail adds on
                    # DVE so they don't pollute the endgame queue
                    r.ins.set_nosync_dependencies(NameSet(
                        list(r.ins.nosync_dependency_names())
                        + [last_max[0].ins.name]
                    ))
                r = nc.vector.tensor_tensor(
                    out=sl, in0=sl, in1=x2[:, 2, :], op=mybir.AluOpType.add,
                )
                NAME_MAP[r.ins.name] = ("add2", ti)
                # count(s > 1) -> acc col ti (4x TS; independent of sqrt)
                cscr = spool.tile([P, M], _BF, tag="c" + sz,
                                  bufs=min(2, m_count[M]))
                r = nc.vector.tensor_scalar(
                    out=cscr, in0=sl, scalar1=1.0, scalar2=None,
                    op0=mybir.AluOpType.is_gt, op1=mybir.AluOpType.add,
                    accum_out=accs[:, ti : ti + 1],
                )
                NAME_MAP[r.ins.name] = ("count", ti)
                count_of[ti] = r
                acc_writers.append(r)

            def stage_b(pi, M2, s2, after=None):
                sz = str(M2)
                use_sqrt_acc = sqrt_acc_last and pi >= n_pairs - sqrt_acc_last
                d = spool.tile([P, M2], _BF, tag="d" + sz,
                               bufs=min(4, m2_count[M2]))
                if use_sqrt_acc:
                    # m = max(s,1) on DVE (4x, right after add2 in-queue),
                    # then ACT sqrt-with-accum: acc += sum(sqrt(m)) =
                    # sum(max(d,1)). Ends on ACT -> no post-sqrt DVE hop.
                    m = spool.tile([P, M2], _BF, tag="m" + sz,
                                   bufs=min(2, m2_count[M2]))
                    r = nc.vector.tensor_scalar(
                        out=m, in0=s2, scalar1=1.0, scalar2=None,
                        op0=mybir.AluOpType.max, op1=mybir.AluOpType.add,
                        accum_out=dummy_acc,
                    )
                    NAME_MAP[r.ins.name] = ("tsmax", pi)
                    tsmax_of[pi] = r
                    r = nc.scalar.activation(
                        out=d, in_=m, func=mybir.ActivationFunctionType.Sqrt,
                        accum_out=accs[:, n_tiles + pi : n_tiles + pi + 1],
                    )
                    NAME_MAP[r.ins.name] = ("sqrt", pi)
                    sqrt_of[pi] = r
                    if after is not None:
                        r.ins.set_nosync_dependencies(NameSet(
                            list(r.ins.nosync_dependency_names())
                            + [after.ins.name]
                        ))
                    acc_writers.append(r)
                    return
                r = nc.scalar.activation(
                    out=d, in_=s2, func=mybir.ActivationFunctionType.Sqrt,
                )
                NAME_MAP[r.ins.name] = ("sqrt", pi)
                sqrt_of[pi] = r
                if after is not None:
                    # scheduler-only edge: keep this sqrt BEHIND the newest
                    # tile's ACT square so squares stay DMA-anchored
                    r.ins.set_nosync_dependencies(NameSet(
                        list(r.ins.nosync_dependency_names())
                        + [after.ins.name]
                    ))
                # sum(max(d,1)) -> acc col n_tiles+pi (4x TS)
                mscr = spool.tile([P, M2], _BF, tag="mx" + sz,
                                  bufs=min(2, m2_count[M2]))
                r = nc.vector.tensor_scalar(
                    out=mscr, in0=d, scalar1=1.0, scalar2=None,
                    op0=mybir.AluOpType.max, op1=mybir.AluOpType.add,
                    accum_out=accs[:, n_tiles + pi : n_tiles + pi + 1],
                )
                NAME_MAP[r.ins.name] = ("max", pi)
                max_of[pi] = r
                acc_writers.append(r)
                last_max[0] = r

            pending = deque()
            for pi, pr in enumerate(plan):
                M2 = sum(pr)
                s2 = spool.tile([P, M2], _BF, tag="s" + str(M2),
                                bufs=min(4, m2_count[M2]))
                s_off = 0
                for M in pr:
                    stage_a(gi[0], M, s2, s_off)
                    s_off += M
                    gi[0] += 1
                pending.append((pi, M2, s2))
                last_pair = pi == n_pairs - 1
                eff_lag = lag
                if pi >= n_pairs - tail_edge:
                    eff_lag = lag + tail_lag_extra
                if last_flush and last_pair:
                    eff_lag = 0
                while len(pending) > eff_lag:
                    if pi >= n_pairs - tail_edge:
                        # tail: order sqrts after the newest ACT square
                        aft = None
                        for tj in range(gi[0] - 1, -1, -1):
                            if sq_act_of.get(tj) is not None:
                                aft = sq_act_of[tj]
                                break
                    else:
                        aft = sq_act_of.get(gi[0] - len(pr))
                    stage_b(*pending.popleft(), after=aft)
            while pending:
                stage_b(*pending.popleft())

            if endgame_edges:
                def add_nosync(ins_r, after_r):
                    nm = after_r.ins.name
                    if (nm in ins_r.ins.sync_dependency_names()
                            or nm in ins_r.ins.nosync_dependency_names()):
                        return
                    ins_r.ins.set_nosync_dependencies(NameSet(
                        list(ins_r.ins.nosync_dependency_names()) + [nm]
                    ))
                # keep late mid-pair maxes out of the endgame DVE chain:
                # order them after the last tile's count
                lc = count_of.get(n_tiles - 1)
                if lc is not None:
                    for pj in range(max(0, n_pairs - 4), n_pairs):
                        if pj in max_of:
                            add_nosync(max_of[pj], lc)
                # let the last counts overlap the final sqrt: order them
                # after the last pair's tsmax on DVE
                tm = tsmax_of.get(n_pairs - 1)
                if tm is not None:
                    for tj in range(max(0, n_tiles - eg_ct), n_tiles):
                        if tj in count_of:
                            add_nosync(count_of[tj], tm)
                # keep the tail sqrts behind the LAST ACT square so the
                # final tile's square is never queued behind them
                last_sq = None
                for tj in range(n_tiles - 1, -1, -1):
                    if sq_act_of.get(tj) is not None:
                        last_sq = sq_act_of[tj]
                        break
                if last_sq is not None:
                    for pj in range(max(0, n_pairs - 1 - eg_sq), n_pairs):
                        if pj in sqrt_of:
                            add_nosync(sqrt_of[pj], last_sq)

            # out-DMA via pre-staged SWDGE descriptors (prep early, trigger
            # after the final accumulate; RAW edges moved to the trigger)
            in_view = accs[:, :].rearrange("p (a b w) -> p a b w", a=1, b=1)
            out_view = out[:, :].rearrange("p (a b w) -> a p b w", a=1, b=1)
            wb_prep = nc.gpsimd.kv_writeback(
                out_view, in_view, wb_idx[:, :], prepare_only=True, sem=wb_sem,
            )
            wb_trig = nc.gpsimd.trigger_dma(count=None)
            acc_names = {w.ins.name for w in acc_writers}
            prep_sync = list(wb_prep.ins.sync_dependency_names())
            wb_prep.ins.set_sync_dependencies(
                NameSet([d for d in prep_sync if d not in acc_names])
            )
            wb_prep.ins.set_nosync_dependencies(NameSet(
                [d for d in wb_prep.ins.nosync_dependency_names()
                 if d not in acc_names]
            ))
            wb_trig.ins.set_sync_dependencies(NameSet(
                list(wb_trig.ins.sync_dependency_names()) + sorted(acc_names)
            ))

    nc.compile()

    # point the prep's on_update[0] at the DMASW drain sem (scatter_add-style
    # wiring; see v1 kernel for rationale)
    dmasw = None
    for i in nc.all_instructions():
        if i.sync_info:
            for w in i.sync_info.on_wait:
                if w.ant_name and w.ant_name.startswith("DMASW"):
                    dmasw = (w.id, w.ant_name)
    assert dmasw is not None, "no DMASW drain wait found"
    wb_prep.ins.sync_info.on_update[0] = mybir.SyncUpdate(
        sync_type="semaphore", id=dmasw[0], ant_name=dmasw[1],
        update_mode="sem-add-imm", update_value=16,
    )
    return nc


NAME_MAP = {}  # ins name -> (kind, index) for trace attribution

_nc_cache = None
last_results = None


def kernel(kps_world_pred: np.ndarray) -> np.ndarray:
    global _nc_cache, last_results
    x = np.ascontiguousarray(kps_world_pred, dtype=np.float32)
    assert x.shape == (B, J, D)

    # shard + deinterleave: [8, P, 8704 triplets, 3] -> [8, P, 3, 8704]
    v = np.ascontiguousarray(
        x.reshape(N_CORES, P, M_TOT, 3).transpose(0, 1, 3, 2)
    )
    in_maps = [{"x": v[c]} for c in range(N_CORES)]

    if _nc_cache is None:
        _nc_cache = build_nc()

    import time

    res = None
    for attempt in range(3):
        try:
            res = run_bass_kernel_spmd(_nc_cache, in_maps, list(range(N_CORES)))
            break
        except Exception:
            if attempt == 2:
                raise
            time.sleep(15)
    last_results = res

    # identity: sum(max(d,1)) + count(s>1) = masked_sum + P*M_TOT per core
    total = np.float64(0.0)
    for c in range(N_CORES):
        total += res.results[c]["out"].astype(np.float64).sum()
    total -= np.float64(N_CORES * P * M_TOT)
    return np.asarray(total / (B * J), dtype=np.float32)


# revision 11
# speedup vs baseline: 1.0083x; 1.0054x over previous
"""Trainium2 Bass kernel for BodyStructureLoss (deinterleaved thirds).

Host deinterleaves each core's shard into [P, 3, 8704]: per partition row
[all x | all y | all z]. Per tile of M norm-columns:
  - one DMA brings [P, 3, M] (three contiguous M-runs per partition)
  - squares run in parallel per third (x->ACT, y->DVE, z->Pool mid-stream;
    the DVE-heavy z moves around in the tail), all fp32 -> bf16
  - s = x2+y2 (+z2) via two packed-bf16 DVE adds (2x mode)
  - count-TS (s>1, DVE 4x) accumulates per tile into its own acc column
Tiles are grouped in PAIRS sharing one s2 buffer; per pair one ACT sqrt
(bf16) + one max-TS (sum(max(d,1)), DVE 4x). Pairing gives the ACT queue
slack against the s->sqrt feedback loop; nosync scheduler edges keep each
sqrt BEHIND the newest tile's ACT square so squares stay DMA-anchored.
The final pair instead runs m=max(s,1) (DVE 4x) then an ACT
sqrt-with-accumulate, ending the critical chain on ACT with no post-sqrt
DVE hop. TS scratch outputs rotate per-op (a shared scratch would WAW-
serialize every accumulate against every other).
Host: sum acc, subtract P*8704 per core (count+max identity:
sum(max(d,1)) + count(s>1) = masked_sum + N), divide by B*J.
"""

import os

import numpy as np

os.environ["BASS_NEVER_TRACE"] = "1"

import concourse.bacc as bacc
import concourse.mybir as mybir
from concourse.bass_utils import run_bass_kernel_spmd
from concourse.tile import TileContext

B, J, D = 524288, 17, 3
N_CORES = 8
P = 128
M_TOT = B // N_CORES * J // P  # 8704 norm columns per partition
_DT = mybir.dt.float32
_BF = mybir.dt.bfloat16

# pairs of tile sizes (norm-columns); a 1-element pair gets its own sqrt
PLAN = [[160, 160]] + [[576, 576]] * 6 + [[336], [336], [176, 176],
                                          [160, 160], [128]]
assert sum(m for pr in PLAN for m in pr) == M_TOT


def build_nc(plan=None, lag=1, xbufs=4, sq_eng=("act", "dve", "pool"),
             tail_dve_sq=0, last_flush=True, tail_pool=0, tail_edge=3,
             tail_eng=("act", "pool", "dve"), tail_eng_n=7,
             last2_eng=("act", "pool", "pool"), tail_lag_extra=0,
             sqrt_acc_last=1, pin_max_tail=0, endgame_edges=True,
             eg_ct=2, eg_sq=2, pair_count_from=9, merge_max_tail=3):
    import bass_rust
    from collections import Counter, deque
    NameSet = bass_rust.InstructionNameOrderedSet

    if plan is None:
        plan = PLAN
    flat = [m for pr in plan for m in pr]
    assert sum(flat) == M_TOT
    n_tiles = len(flat)
    n_pairs = len(plan)
    m_count = Counter(flat)
    m2_count = Counter(sum(pr) for pr in plan)

    nc = bacc.Bacc(
        "TRN2", target_bir_lowering=False, debug=False, num_devices=N_CORES
    )
    x = nc.dram_tensor("x", [P, 3, M_TOT], _DT, kind="ExternalInput")
    # acc columns: one count col per tile + one max col per pair
    acc_cols = n_tiles + n_pairs
    out = nc.dram_tensor("out", [P, acc_cols], _DT, kind="ExternalOutput")

    with TileContext(nc) as tc:
        with (
            tc.tile_pool(name="xin", bufs=xbufs) as xpool,
            tc.tile_pool(name="small", bufs=4) as spool,
            tc.tile_pool(name="accp", bufs=1) as accpool,
        ):
            eng_of = {"dve": nc.vector, "pool": nc.gpsimd}
            accs = accpool.tile([P, acc_cols], _DT)
            # zero acc so unused columns (merged-max mode) sum to 0 on host
            nc.vector.memset(accs, 0.0)
            merged_pairs = set()
            if merge_max_tail:
                lo = n_pairs - sqrt_acc_last - merge_max_tail
                merged_pairs = set(range(max(0, lo), n_pairs - sqrt_acc_last))
            merged_m2 = sum(sum(plan[pj]) for pj in merged_pairs)
            if merged_m2:
                dmerge = accpool.tile([P, merged_m2], _BF, tag="dmerge")
            else:
                dmerge = None
            dm_off = [0]
            dm_state = {"col": None, "deps": []}
            scr16 = accpool.tile([P, 2], _BF)  # dummy-sqrt operand only

            # dummy sqrt: preload the ACT table set containing Sqrt+Square
            nc.vector.memset(scr16[:, :1], 1.0)
            nc.scalar.activation(
                out=scr16[:, :1], in_=scr16[:, :1],
                func=mybir.ActivationFunctionType.Sqrt,
            )

            acc_writers = []
            wb_idx = accpool.tile([P, 1], mybir.dt.int32)
            nc.gpsimd.memset(wb_idx, 0)
            wb_sem = nc.alloc_semaphore("wb_dma")
            # scratch accum target: walrus rejects tensor_scalar without an
            # accum_out, so the elementwise max accumulates here (unused)
            dummy_acc = accpool.tile([P, 1], _DT)

            gi = [0]  # global tile index
            m_off = [0]
            sq_act_of = {}  # tile -> its ACT square handle (or None)
            last_max = [None]  # most recent max/tsmax handle (DVE)
            count_of = {}  # tile -> count handle
            tsmax_of = {}  # pair -> tsmax handle (sqrt_acc path)
            max_of = {}  # pair -> max handle (regular path)
            sqrt_of = {}  # pair -> sqrt handle

            def stage_a(ti, M, s2, s_off, pi=None, last_in_pair=True):
                sz = str(M)
                nb = min(xbufs, m_count[M])
                xt = xpool.tile([P, 3, M], _DT, tag="xt" + sz, bufs=nb)
                m0 = m_off[0]
                r = nc.sync.dma_start(out=xt, in_=x[:, :, m0 : m0 + M])
                NAME_MAP[r.ins.name] = ("dma", ti)
                m_off[0] += M
                x2 = spool.tile([P, 3, M], _BF, tag="x2" + sz,
                                bufs=min(4, m_count[M]))
                ndve = tail_dve_sq and ti >= n_tiles - tail_dve_sq
                npool = tail_pool and ti >= n_tiles - tail_pool
                engs = sq_eng
                if tail_eng and ti >= n_tiles - tail_eng_n:
                    engs = tail_eng
                if last2_eng and ti >= n_tiles - 2:
                    engs = last2_eng
                sq_act = None
                for k, ename in enumerate(engs):
                    if ndve:
                        ename = "dve"
                    elif npool and k > 0:
                        # tail: y,z squares on Pool to unload DVE's queue
                        ename = "pool"
                    if ename == "act":
                        r = nc.scalar.activation(
                            out=x2[:, k, :], in_=xt[:, k, :],
                            func=mybir.ActivationFunctionType.Square,
                        )
                        sq_act = r
                    else:
                        r = eng_of[ename].tensor_tensor(
                            out=x2[:, k, :], in0=xt[:, k, :], in1=xt[:, k, :],
                            op=mybir.AluOpType.mult,
                        )
                    NAME_MAP[r.ins.name] = ("sq_" + "xyz"[k], ti)
                sq_act_of[ti] = sq_act
                sl = s2[:, s_off : s_off + M]
                r = nc.vector.tensor_tensor(
                    out=sl, in0=x2[:, 0, :], in1=x2[:, 1, :],
                    op=mybir.AluOpType.add,
                )
                NAME_MAP[r.ins.name] = ("add1", ti)
                if (pin_max_tail and ti >= n_tiles - pin_max_tail
                        and last_max[0] is not None):
                    # keep earlier pairs' max ops AHEAD of the tail adds on
                    # DVE so they don't pollute the endgame queue
                    r.ins.set_nosync_dependencies(NameSet(
                        list(r.ins.nosync_dependency_names())
                        + [last_max[0].ins.name]
                    ))
                r = nc.vector.tensor_tensor(
                    out=sl, in0=sl, in1=x2[:, 2, :], op=mybir.AluOpType.add,
                )
                NAME_MAP[r.ins.name] = ("add2", ti)
                # count(s > 1) -> acc col ti (4x TS; independent of sqrt).
                # pair-counted pairs emit ONE count over the full s2 on the
                # second tile (fewer endgame DVE ops).
                paired_ct = (pair_count_from is not None and pi is not None
                             and pi >= pair_count_from)
                if paired_ct and not last_in_pair:
                    return
                cin = s2[:, : s_off + M] if paired_ct else sl
                csz = s_off + M if paired_ct else M
                cscr = spool.tile([P, csz], _BF, tag="c" + str(csz),
                                  bufs=min(2, max(m_count[M], 1)))
                r = nc.vector.tensor_scalar(
                    out=cscr, in0=cin, scalar1=1.0, scalar2=None,
                    op0=mybir.AluOpType.is_gt, op1=mybir.AluOpType.add,
                    accum_out=accs[:, ti : ti + 1],
                )
                NAME_MAP[r.ins.name] = ("count", ti)
                count_of[ti] = r
                acc_writers.append(r)

            def stage_b(pi, M2, s2, after=None):
                sz = str(M2)
                use_sqrt_acc = sqrt_acc_last and pi >= n_pairs - sqrt_acc_last
                if pi in merged_pairs:
                    # write d into the shared merge buffer; ONE max op over
                    # the whole buffer is emitted after the loop
                    d = dmerge[:, dm_off[0] : dm_off[0] + M2]
                    dm_off[0] += M2
                    r = nc.scalar.activation(
                        out=d, in_=s2,
                        func=mybir.ActivationFunctionType.Sqrt,
                    )
                    NAME_MAP[r.ins.name] = ("sqrt", pi)
                    sqrt_of[pi] = r
                    if after is not None:
                        r.ins.set_nosync_dependencies(NameSet(
                            list(r.ins.nosync_dependency_names())
                            + [after.ins.name]
                        ))
                    if dm_state["col"] is None:
                        dm_state["col"] = n_tiles + pi
                    return
                d = spool.tile([P, M2], _BF, tag="d" + sz,
                               bufs=min(4, m2_count[M2]))
                if use_sqrt_acc:
                    # m = max(s,1) on DVE (4x, right after add2 in-queue),
                    # then ACT sqrt-with-accum: acc += sum(sqrt(m)) =
                    # sum(max(d,1)). Ends on ACT -> no post-sqrt DVE hop.
                    m = spool.tile([P, M2], _BF, tag="m" + sz,
                                   bufs=min(2, m2_count[M2]))
                    r = nc.vector.tensor_scalar(
                        out=m, in0=s2, scalar1=1.0, scalar2=None,
                        op0=mybir.AluOpType.max, op1=mybir.AluOpType.add,
                        accum_out=dummy_acc,
                    )
                    NAME_MAP[r.ins.name] = ("tsmax", pi)
                    tsmax_of[pi] = r
                    r = nc.scalar.activation(
                        out=d, in_=m, func=mybir.ActivationFunctionType.Sqrt,
                        accum_out=accs[:, n_tiles + pi : n_tiles + pi + 1],
                    )
                    NAME_MAP[r.ins.name] = ("sqrt", pi)
                    sqrt_of[pi] = r
                    if after is not None:
                        r.ins.set_nosync_dependencies(NameSet(
                            list(r.ins.nosync_dependency_names())
                            + [after.ins.name]
                        ))
                    acc_writers.append(r)
                    return
                r = nc.scalar.activation(
                    out=d, in_=s2, func=mybir.ActivationFunctionType.Sqrt,
                )
                NAME_MAP[r.ins.name] = ("sqrt", pi)
                sqrt_of[pi] = r
                if after is not None:
                    # scheduler-only edge: keep this sqrt BEHIND the newest
                    # tile's ACT square so squares stay DMA-anchored
                    r.ins.set_nosync_dependencies(NameSet(
                        list(r.ins.nosync_dependency_names())
                        + [after.ins.name]
                    ))
                # sum(max(d,1)) -> acc col n_tiles+pi (4x TS)
                mscr = spool.tile([P, M2], _BF, tag="mx" + sz,
                                  bufs=min(2, m2_count[M2]))
                r = nc.vector.tensor_scalar(
                    out=mscr, in0=d, scalar1=1.0, scalar2=None,
                    op0=mybir.AluOpType.max, op1=mybir.AluOpType.add,
                    accum_out=accs[:, n_tiles + pi : n_tiles + pi + 1],
                )
                NAME_MAP[r.ins.name] = ("max", pi)
                max_of[pi] = r
                acc_writers.append(r)
                last_max[0] = r

            pending = deque()
            for pi, pr in enumerate(plan):
                M2 = sum(pr)
                s2 = spool.tile([P, M2], _BF, tag="s" + str(M2),
                                bufs=min(4, m2_count[M2]))
                s_off = 0
                for mi, M in enumerate(pr):
                    stage_a(gi[0], M, s2, s_off, pi=pi,
                            last_in_pair=(mi == len(pr) - 1))
                    s_off += M
                    gi[0] += 1
                pending.append((pi, M2, s2))
                last_pair = pi == n_pairs - 1
                eff_lag = lag
                if pi >= n_pairs - tail_edge:
                    eff_lag = lag + tail_lag_extra
                if last_flush and last_pair:
                    eff_lag = 0
                while len(pending) > eff_lag:
                    if pi >= n_pairs - tail_edge:
                        # tail: order sqrts after the newest ACT square
                        aft = None
                        for tj in range(gi[0] - 1, -1, -1):
                            if sq_act_of.get(tj) is not None:
                                aft = sq_act_of[tj]
                                break
                    else:
                        aft = sq_act_of.get(gi[0] - len(pr))
                    stage_b(*pending.popleft(), after=aft)
            while pending:
                stage_b(*pending.popleft())

            if dmerge is not None and dm_state["col"] is not None:
                mscr = accpool.tile([P, merged_m2], _BF, tag="mscr")
                r = nc.vector.tensor_scalar(
                    out=mscr, in0=dmerge, scalar1=1.0, scalar2=None,
                    op0=mybir.AluOpType.max, op1=mybir.AluOpType.add,
                    accum_out=accs[:, dm_state["col"] : dm_state["col"] + 1],
                )
                NAME_MAP[r.ins.name] = ("maxM", 0)
                acc_writers.append(r)

            if endgame_edges:
                def add_nosync(ins_r, after_r):
                    nm = after_r.ins.name
                    if (nm in ins_r.ins.sync_dependency_names()
                            or nm in ins_r.ins.nosync_dependency_names()):
                        return
                    ins_r.ins.set_nosync_dependencies(NameSet(
                        list(ins_r.ins.nosync_dependency_names()) + [nm]
                    ))
                # keep late mid-pair maxes out of the endgame DVE chain:
                # order them after the last tile's count
                lc = count_of.get(n_tiles - 1)
                if lc is not None:
                    for pj in range(max(0, n_pairs - 4), n_pairs):
                        if pj in max_of:
                            add_nosync(max_of[pj], lc)
                # let the last counts overlap the final sqrt: order them
                # after the last pair's tsmax on DVE
                tm = tsmax_of.get(n_pairs - 1)
                if tm is not None:
                    for tj in range(max(0, n_tiles - eg_ct), n_tiles):
                        if tj in count_of:
                            add_nosync(count_of[tj], tm)
                # keep the tail sqrts behind the LAST ACT square so the
                # final tile's square is never queued behind them
                last_sq = None
                for tj in range(n_tiles - 1, -1, -1):
                    if sq_act_of.get(tj) is not None:
                        last_sq = sq_act_of[tj]
                        break
                if last_sq is not None:
                    for pj in range(max(0, n_pairs - 1 - eg_sq), n_pairs):
                        if pj in sqrt_of:
                            add_nosync(sqrt_of[pj], last_sq)

            # out-DMA via pre-staged SWDGE descriptors (prep early, trigger
            # after the final accumulate; RAW edges moved to the trigger)
            in_view = accs[:, :].rearrange("p (a b w) -> p a b w", a=1, b=1)
            out_view = out[:, :].rearrange("p (a b w) -> a p b w", a=1, b=1)
            wb_prep = nc.gpsimd.kv_writeback(
                out_view, in_view, wb_idx[:, :], prepare_only=True, sem=wb_sem,
            )
            wb_trig = nc.gpsimd.trigger_dma(count=None)
            acc_names = {w.ins.name for w in acc_writers}
            prep_sync = list(wb_prep.ins.sync_dependency_names())
            wb_prep.ins.set_sync_dependencies(
                NameSet([d for d in prep_sync if d not in acc_names])
            )
            wb_prep.ins.set_nosync_dependencies(NameSet(
                [d for d in wb_prep.ins.nosync_dependency_names()
                 if d not in acc_names]
            ))
            wb_trig.ins.set_sync_dependencies(NameSet(
                list(wb_trig.ins.sync_dependency_names()) + sorted(acc_names)
            ))

    nc.compile()

    # point the prep's on_update[0] at the DMASW drain sem (scatter_add-style
    # wiring; see v1 kernel for rationale)
    dmasw = None
    for i in nc.all_instructions():
        if i.sync_info:
            for w in i.sync_info.on_wait:
                if w.ant_name and w.ant_name.startswith("DMASW"):
                    dmasw = (w.id, w.ant_name)
    assert dmasw is not None, "no DMASW drain wait found"
    wb_prep.ins.sync_info.on_update[0] = mybir.SyncUpdate(
        sync_type="semaphore", id=dmasw[0], ant_name=dmasw[1],
        update_mode="sem-add-imm", update_value=16,
    )
    return nc


NAME_MAP = {}  # ins name -> (kind, index) for trace attribution

_nc_cache = None
last_results = None


def kernel(kps_world_pred: np.ndarray) -> np.ndarray:
    global _nc_cache, last_results
    x = np.ascontiguousarray(kps_world_pred, dtype=np.float32)
    assert x.shape == (B, J, D)

    # shard + deinterleave: [8, P, 8704 triplets, 3] -> [8, P, 3, 8704]
    v = np.ascontiguousarray(
        x.reshape(N_CORES, P, M_TOT, 3).transpose(0, 1, 3, 2)
    )
    in_maps = [{"x": v[c]} for c in range(N_CORES)]

    if _nc_cache is None:
        _nc_cache = build_nc()

    import time

    res = None
    for attempt in range(3):
        try:
            res = run_bass_kernel_spmd(_nc_cache, in_maps, list(range(N_CORES)))
            break
        except Exception:
            if attempt == 2:
                raise
            time.sleep(15)
    last_results = res

    # identity: sum(max(d,1)) + count(s>1) = masked_sum + P*M_TOT per core
    total = np.float64(0.0)
    for c in range(N_CORES):
        total += res.results[c]["out"].astype(np.float64).sum()
    total -= np.float64(N_CORES * P * M_TOT)
    return np.asarray(total / (B * J), dtype=np.float32)


# revision 12
# speedup vs baseline: 1.0088x; 1.0005x over previous
"""Trainium2 Bass kernel for BodyStructureLoss (deinterleaved thirds).

Host deinterleaves each core's shard into [P, 3, 8704]: per partition row
[all x | all y | all z]. Per tile of M norm-columns:
  - one DMA brings [P, 3, M] (three contiguous M-runs per partition)
  - squares run in parallel per third (x->ACT, y->DVE, z->Pool mid-stream;
    the DVE-heavy z moves around in the tail), all fp32 -> bf16
  - s = x2+y2 (+z2) via two packed-bf16 DVE adds (2x mode)
  - count-TS (s>1, DVE 4x) accumulates per tile into its own acc column
Tiles are grouped in PAIRS sharing one s2 buffer; per pair one ACT sqrt
(bf16) + one max-TS (sum(max(d,1)), DVE 4x). Pairing gives the ACT queue
slack against the s->sqrt feedback loop; nosync scheduler edges keep each
sqrt BEHIND the newest tile's ACT square so squares stay DMA-anchored.
The final pair instead runs m=max(s,1) (DVE 4x) then an ACT
sqrt-with-accumulate, ending the critical chain on ACT with no post-sqrt
DVE hop. TS scratch outputs rotate per-op (a shared scratch would WAW-
serialize every accumulate against every other).
Host: sum acc, subtract P*8704 per core (count+max identity:
sum(max(d,1)) + count(s>1) = masked_sum + N), divide by B*J.
"""

import os

import numpy as np

os.environ["BASS_NEVER_TRACE"] = "1"

import concourse.bacc as bacc
import concourse.mybir as mybir
from concourse.bass_utils import run_bass_kernel_spmd
from concourse.tile import TileContext

B, J, D = 524288, 17, 3
N_CORES = 8
P = 128
M_TOT = B // N_CORES * J // P  # 8704 norm columns per partition
_DT = mybir.dt.float32
_BF = mybir.dt.bfloat16

# pairs of tile sizes (norm-columns); a 1-element pair gets its own sqrt
PLAN = [[160, 160]] + [[576, 576]] * 6 + [[336], [336], [168, 168],
                                          [152, 152], [160]]
assert sum(m for pr in PLAN for m in pr) == M_TOT


def build_nc(plan=None, lag=1, xbufs=4, sq_eng=("act", "dve", "pool"),
             tail_dve_sq=0, last_flush=True, tail_pool=0, tail_edge=3,
             tail_eng=("act", "pool", "dve"), tail_eng_n=7,
             last2_eng=("act", "pool", "pool"), tail_lag_extra=0,
             sqrt_acc_last=1, pin_max_tail=0, endgame_edges=True,
             eg_ct=2, eg_sq=2, pair_count_from=9, merge_max_tail=3):
    import bass_rust
    from collections import Counter, deque
    NameSet = bass_rust.InstructionNameOrderedSet

    if plan is None:
        plan = PLAN
    flat = [m for pr in plan for m in pr]
    assert sum(flat) == M_TOT
    n_tiles = len(flat)
    n_pairs = len(plan)
    m_count = Counter(flat)
    m2_count = Counter(sum(pr) for pr in plan)

    nc = bacc.Bacc(
        "TRN2", target_bir_lowering=False, debug=False, num_devices=N_CORES
    )
    x = nc.dram_tensor("x", [P, 3, M_TOT], _DT, kind="ExternalInput")
    # acc columns: one count col per tile + one max col per pair
    acc_cols = n_tiles + n_pairs
    out = nc.dram_tensor("out", [P, acc_cols], _DT, kind="ExternalOutput")

    with TileContext(nc) as tc:
        with (
            tc.tile_pool(name="xin", bufs=xbufs) as xpool,
            tc.tile_pool(name="small", bufs=4) as spool,
            tc.tile_pool(name="accp", bufs=1) as accpool,
        ):
            eng_of = {"dve": nc.vector, "pool": nc.gpsimd}
            accs = accpool.tile([P, acc_cols], _DT)
            # zero acc so unused columns (merged-max mode) sum to 0 on host
            nc.vector.memset(accs, 0.0)
            merged_pairs = set()
            if merge_max_tail:
                lo = n_pairs - sqrt_acc_last - merge_max_tail
                merged_pairs = set(range(max(0, lo), n_pairs - sqrt_acc_last))
            merged_m2 = sum(sum(plan[pj]) for pj in merged_pairs)
            if merged_m2:
                dmerge = accpool.tile([P, merged_m2], _BF, tag="dmerge")
            else:
                dmerge = None
            dm_off = [0]
            dm_state = {"col": None, "deps": []}
            scr16 = accpool.tile([P, 2], _BF)  # dummy-sqrt operand only

            # dummy sqrt: preload the ACT table set containing Sqrt+Square
            nc.vector.memset(scr16[:, :1], 1.0)
            nc.scalar.activation(
                out=scr16[:, :1], in_=scr16[:, :1],
                func=mybir.ActivationFunctionType.Sqrt,
            )

            acc_writers = []
            wb_idx = accpool.tile([P, 1], mybir.dt.int32)
            nc.gpsimd.memset(wb_idx, 0)
            wb_sem = nc.alloc_semaphore("wb_dma")
            # scratch accum target: walrus rejects tensor_scalar without an
            # accum_out, so the elementwise max accumulates here (unused)
            dummy_acc = accpool.tile([P, 1], _DT)

            gi = [0]  # global tile index
            m_off = [0]
            sq_act_of = {}  # tile -> its ACT square handle (or None)
            last_max = [None]  # most recent max/tsmax handle (DVE)
            count_of = {}  # tile -> count handle
            tsmax_of = {}  # pair -> tsmax handle (sqrt_acc path)
            max_of = {}  # pair -> max handle (regular path)
            sqrt_of = {}  # pair -> sqrt handle

            def stage_a(ti, M, s2, s_off, pi=None, last_in_pair=True):
                sz = str(M)
                nb = min(xbufs, m_count[M])
                xt = xpool.tile([P, 3, M], _DT, tag="xt" + sz, bufs=nb)
                m0 = m_off[0]
                r = nc.sync.dma_start(out=xt, in_=x[:, :, m0 : m0 + M])
                NAME_MAP[r.ins.name] = ("dma", ti)
                m_off[0] += M
                x2 = spool.tile([P, 3, M], _BF, tag="x2" + sz,
                                bufs=min(4, m_count[M]))
                ndve = tail_dve_sq and ti >= n_tiles - tail_dve_sq
                npool = tail_pool and ti >= n_tiles - tail_pool
                engs = sq_eng
                if tail_eng and ti >= n_tiles - tail_eng_n:
                    engs = tail_eng
                if last2_eng and ti >= n_tiles - 2:
                    engs = last2_eng
                sq_act = None
                for k, ename in enumerate(engs):
                    if ndve:
                        ename = "dve"
                    elif npool and k > 0:
                        # tail: y,z squares on Pool to unload DVE's queue
                        ename = "pool"
                    if ename == "act":
                        r = nc.scalar.activation(
                            out=x2[:, k, :], in_=xt[:, k, :],
                            func=mybir.ActivationFunctionType.Square,
                        )
                        sq_act = r
                    else:
                        r = eng_of[ename].tensor_tensor(
                            out=x2[:, k, :], in0=xt[:, k, :], in1=xt[:, k, :],
                            op=mybir.AluOpType.mult,
                        )
                    NAME_MAP[r.ins.name] = ("sq_" + "xyz"[k], ti)
                sq_act_of[ti] = sq_act
                sl = s2[:, s_off : s_off + M]
                r = nc.vector.tensor_tensor(
                    out=sl, in0=x2[:, 0, :], in1=x2[:, 1, :],
                    op=mybir.AluOpType.add,
                )
                NAME_MAP[r.ins.name] = ("add1", ti)
                if (pin_max_tail and ti >= n_tiles - pin_max_tail
                        and last_max[0] is not None):
                    # keep earlier pairs' max ops AHEAD of the tail adds on
                    # DVE so they don't pollute the endgame queue
                    r.ins.set_nosync_dependencies(NameSet(
                        list(r.ins.nosync_dependency_names())
                        + [last_max[0].ins.name]
                    ))
                r = nc.vector.tensor_tensor(
                    out=sl, in0=sl, in1=x2[:, 2, :], op=mybir.AluOpType.add,
                )
                NAME_MAP[r.ins.name] = ("add2", ti)
                # count(s > 1) -> acc col ti (4x TS; independent of sqrt).
                # pair-counted pairs emit ONE count over the full s2 on the
                # second tile (fewer endgame DVE ops).
                paired_ct = (pair_count_from is not None and pi is not None
                             and pi >= pair_count_from)
                if paired_ct and not last_in_pair:
                    return
                cin = s2[:, : s_off + M] if paired_ct else sl
                csz = s_off + M if paired_ct else M
                cscr = spool.tile([P, csz], _BF, tag="c" + str(csz),
                                  bufs=min(2, max(m_count[M], 1)))
                r = nc.vector.tensor_scalar(
                    out=cscr, in0=cin, scalar1=1.0, scalar2=None,
                    op0=mybir.AluOpType.is_gt, op1=mybir.AluOpType.add,
                    accum_out=accs[:, ti : ti + 1],
                )
                NAME_MAP[r.ins.name] = ("count", ti)
                count_of[ti] = r
                acc_writers.append(r)

            def stage_b(pi, M2, s2, after=None):
                sz = str(M2)
                use_sqrt_acc = sqrt_acc_last and pi >= n_pairs - sqrt_acc_last
                if pi in merged_pairs:
                    # write d into the shared merge buffer; ONE max op over
                    # the whole buffer is emitted after the loop
                    d = dmerge[:, dm_off[0] : dm_off[0] + M2]
                    dm_off[0] += M2
                    r = nc.scalar.activation(
                        out=d, in_=s2,
                        func=mybir.ActivationFunctionType.Sqrt,
                    )
                    NAME_MAP[r.ins.name] = ("sqrt", pi)
                    sqrt_of[pi] = r
                    if after is not None:
                        r.ins.set_nosync_dependencies(NameSet(
                            list(r.ins.nosync_dependency_names())
                            + [after.ins.name]
                        ))
                    if dm_state["col"] is None:
                        dm_state["col"] = n_tiles + pi
                    return
                d = spool.tile([P, M2], _BF, tag="d" + sz,
                               bufs=min(4, m2_count[M2]))
                if use_sqrt_acc:
                    # m = max(s,1) on DVE (4x, right after add2 in-queue),
                    # then ACT sqrt-with-accum: acc += sum(sqrt(m)) =
                    # sum(max(d,1)). Ends on ACT -> no post-sqrt DVE hop.
                    m = spool.tile([P, M2], _BF, tag="m" + sz,
                                   bufs=min(2, m2_count[M2]))
                    r = nc.vector.tensor_scalar(
                        out=m, in0=s2, scalar1=1.0, scalar2=None,
                        op0=mybir.AluOpType.max, op1=mybir.AluOpType.add,
                        accum_out=dummy_acc,
                    )
                    NAME_MAP[r.ins.name] = ("tsmax", pi)
                    tsmax_of[pi] = r
                    r = nc.scalar.activation(
                        out=d, in_=m, func=mybir.ActivationFunctionType.Sqrt,
                        accum_out=accs[:, n_tiles + pi : n_tiles + pi + 1],
                    )
                    NAME_MAP[r.ins.name] = ("sqrt", pi)
                    sqrt_of[pi] = r
                    if after is not None:
                        r.ins.set_nosync_dependencies(NameSet(
                            list(r.ins.nosync_dependency_names())
                            + [after.ins.name]
                        ))
                    acc_writers.append(r)
                    return
                r = nc.scalar.activation(
                    out=d, in_=s2, func=mybir.ActivationFunctionType.Sqrt,
                )
                NAME_MAP[r.ins.name] = ("sqrt", pi)
                sqrt_of[pi] = r
                if after is not None:
                    # scheduler-only edge: keep this sqrt BEHIND the newest
                    # tile's ACT square so squares stay DMA-anchored
                    r.ins.set_nosync_dependencies(NameSet(
                        list(r.ins.nosync_dependency_names())
                        + [after.ins.name]
                    ))
                # sum(max(d,1)) -> acc col n_tiles+pi (4x TS)
                mscr = spool.tile([P, M2], _BF, tag="mx" + sz,
                                  bufs=min(2, m2_count[M2]))
                r = nc.vector.tensor_scalar(
                    out=mscr, in0=d, scalar1=1.0, scalar2=None,
                    op0=mybir.AluOpType.max, op1=mybir.AluOpType.add,
                    accum_out=accs[:, n_tiles + pi : n_tiles + pi + 1],
                )
                NAME_MAP[r.ins.name] = ("max", pi)
                max_of[pi] = r
                acc_writers.append(r)
                last_max[0] = r

            pending = deque()
            for pi, pr in enumerate(plan):
                M2 = sum(pr)
                s2 = spool.tile([P, M2], _BF, tag="s" + str(M2),
                                bufs=min(4, m2_count[M2]))
                s_off = 0
                for mi, M in enumerate(pr):
                    stage_a(gi[0], M, s2, s_off, pi=pi,
                            last_in_pair=(mi == len(pr) - 1))
                    s_off += M
                    gi[0] += 1
                pending.append((pi, M2, s2))
                last_pair = pi == n_pairs - 1
                eff_lag = lag
                if pi >= n_pairs - tail_edge:
                    eff_lag = lag + tail_lag_extra
                if last_flush and last_pair:
                    eff_lag = 0
                while len(pending) > eff_lag:
                    if pi >= n_pairs - tail_edge:
                        # tail: order sqrts after the newest ACT square
                        aft = None
                        for tj in range(gi[0] - 1, -1, -1):
                            if sq_act_of.get(tj) is not None:
                                aft = sq_act_of[tj]
                                break
                    else:
                        aft = sq_act_of.get(gi[0] - len(pr))
                    stage_b(*pending.popleft(), after=aft)
            while pending:
                stage_b(*pending.popleft())

            if dmerge is not None and dm_state["col"] is not None:
                mscr = accpool.tile([P, merged_m2], _BF, tag="mscr")
                r = nc.vector.tensor_scalar(
                    out=mscr, in0=dmerge, scalar1=1.0, scalar2=None,
                    op0=mybir.AluOpType.max, op1=mybir.AluOpType.add,
                    accum_out=accs[:, dm_state["col"] : dm_state["col"] + 1],
                )
                NAME_MAP[r.ins.name] = ("maxM", 0)
                acc_writers.append(r)

            if endgame_edges:
                def add_nosync(ins_r, after_r):
                    nm = after_r.ins.name
                    if (nm in ins_r.ins.sync_dependency_names()
                            or nm in ins_r.ins.nosync_dependency_names()):
                        return
                    ins_r.ins.set_nosync_dependencies(NameSet(
                        list(ins_r.ins.nosync_dependency_names()) + [nm]
                    ))
                # keep late mid-pair maxes out of the endgame DVE chain:
                # order them after the last tile's count
                lc = count_of.get(n_tiles - 1)
                if lc is not None:
                    for pj in range(max(0, n_pairs - 4), n_pairs):
                        if pj in max_of:
                            add_nosync(max_of[pj], lc)
                # let the last counts overlap the final sqrt: order them
                # after the last pair's tsmax on DVE
                tm = tsmax_of.get(n_pairs - 1)
                if tm is not None:
                    for tj in range(max(0, n_tiles - eg_ct), n_tiles):
                        if tj in count_of:
                            add_nosync(count_of[tj], tm)
                # keep the tail sqrts behind the LAST ACT square so the
                # final tile's square is never queued behind them
                last_sq = None
                for tj in range(n_tiles - 1, -1, -1):
                    if sq_act_of.get(tj) is not None:
                        last_sq = sq_act_of[tj]
                        break
                if last_sq is not None:
                    for pj in range(max(0, n_pairs - 1 - eg_sq), n_pairs):
                        if pj in sqrt_of:
                            add_nosync(sqrt_of[pj], last_sq)

            # out-DMA via pre-staged SWDGE descriptors (prep early, trigger
            # after the final accumulate; RAW edges moved to the trigger)
            in_view = accs[:, :].rearrange("p (a b w) -> p a b w", a=1, b=1)
            out_view = out[:, :].rearrange("p (a b w) -> a p b w", a=1, b=1)
            wb_prep = nc.gpsimd.kv_writeback(
                out_view, in_view, wb_idx[:, :], prepare_only=True, sem=wb_sem,
            )
            wb_trig = nc.gpsimd.trigger_dma(count=None)
            acc_names = {w.ins.name for w in acc_writers}
            prep_sync = list(wb_prep.ins.sync_dependency_names())
            wb_prep.ins.set_sync_dependencies(
                NameSet([d for d in prep_sync if d not in acc_names])
            )
            wb_prep.ins.set_nosync_dependencies(NameSet(
                [d for d in wb_prep.ins.nosync_dependency_names()
                 if d not in acc_names]
            ))
            wb_trig.ins.set_sync_dependencies(NameSet(
                list(wb_trig.ins.sync_dependency_names()) + sorted(acc_names)
            ))

    nc.compile()

    # point the prep's on_update[0] at the DMASW drain sem (scatter_add-style
    # wiring; see v1 kernel for rationale)
    dmasw = None
    for i in nc.all_instructions():
        if i.sync_info:
            for w in i.sync_info.on_wait:
                if w.ant_name and w.ant_name.startswith("DMASW"):
                    dmasw = (w.id, w.ant_name)
    assert dmasw is not None, "no DMASW drain wait found"
    wb_prep.ins.sync_info.on_update[0] = mybir.SyncUpdate(
        sync_type="semaphore", id=dmasw[0], ant_name=dmasw[1],
        update_mode="sem-add-imm", update_value=16,
    )
    return nc


NAME_MAP = {}  # ins name -> (kind, index) for trace attribution

_nc_cache = None
last_results = None


def kernel(kps_world_pred: np.ndarray) -> np.ndarray:
    global _nc_cache, last_results
    x = np.ascontiguousarray(kps_world_pred, dtype=np.float32)
    assert x.shape == (B, J, D)

    # shard + deinterleave: [8, P, 8704 triplets, 3] -> [8, P, 3, 8704]
    v = np.ascontiguousarray(
        x.reshape(N_CORES, P, M_TOT, 3).transpose(0, 1, 3, 2)
    )
    in_maps = [{"x": v[c]} for c in range(N_CORES)]

    if _nc_cache is None:
        _nc_cache = build_nc()

    import time

    res = None
    for attempt in range(3):
        try:
            res = run_bass_kernel_spmd(_nc_cache, in_maps, list(range(N_CORES)))
            break
        except Exception:
            if attempt == 2:
                raise
            time.sleep(15)
    last_results = res

    # identity: sum(max(d,1)) + count(s>1) = masked_sum + P*M_TOT per core
    total = np.float64(0.0)
    for c in range(N_CORES):
        total += res.results[c]["out"].astype(np.float64).sum()
    total -= np.float64(N_CORES * P * M_TOT)
    return np.asarray(total / (B * J), dtype=np.float32)


# revision 13
# speedup vs baseline: 1.0107x; 1.0018x over previous
"""Trainium2 Bass kernel for BodyStructureLoss (deinterleaved thirds).

Host deinterleaves each core's shard into [P, 3, 8704]: per partition row
[all x | all y | all z]. Per tile of M norm-columns:
  - one DMA brings [P, 3, M] (three contiguous M-runs per partition)
  - squares run in parallel per third (x->ACT, y->DVE, z->Pool mid-stream;
    the DVE-heavy z moves around in the tail), all fp32 -> bf16
  - s = x2+y2 (+z2) via two packed-bf16 DVE adds (2x mode)
  - count-TS (s>1, DVE 4x) accumulates per tile into its own acc column
Tiles are grouped in PAIRS sharing one s2 buffer; per pair one ACT sqrt
(bf16) + one max-TS (sum(max(d,1)), DVE 4x). Pairing gives the ACT queue
slack against the s->sqrt feedback loop; nosync scheduler edges keep each
sqrt BEHIND the newest tile's ACT square so squares stay DMA-anchored.
The final pair instead runs m=max(s,1) (DVE 4x) then an ACT
sqrt-with-accumulate, ending the critical chain on ACT with no post-sqrt
DVE hop. TS scratch outputs rotate per-op (a shared scratch would WAW-
serialize every accumulate against every other).
Host: sum acc, subtract P*8704 per core (count+max identity:
sum(max(d,1)) + count(s>1) = masked_sum + N), divide by B*J.
"""

import os

import numpy as np

os.environ["BASS_NEVER_TRACE"] = "1"

import concourse.bacc as bacc
import concourse.mybir as mybir
from concourse.bass_utils import run_bass_kernel_spmd
from concourse.tile import TileContext

B, J, D = 524288, 17, 3
N_CORES = 8
P = 128
M_TOT = B // N_CORES * J // P  # 8704 norm columns per partition
_DT = mybir.dt.float32
_BF = mybir.dt.bfloat16

# pairs of tile sizes (norm-columns); a 1-element pair gets its own sqrt
PLAN = [[160, 160]] + [[576, 576]] * 6 + [[320], [288], [200, 200],
                                          [152, 152], [160]]
assert sum(m for pr in PLAN for m in pr) == M_TOT


def build_nc(plan=None, lag=1, xbufs=4, sq_eng=("act", "dve", "pool"),
             tail_dve_sq=0, last_flush=True, tail_pool=0, tail_edge=3,
             tail_eng=("act", "pool", "dve"), tail_eng_n=7,
             last2_eng=("act", "pool", "pool"), tail_lag_extra=0,
             sqrt_acc_last=1, pin_max_tail=0, endgame_edges=True,
             eg_ct=2, eg_sq=2, pair_count_from=9, merge_max_tail=3):
    import bass_rust
    from collections import Counter, deque
    NameSet = bass_rust.InstructionNameOrderedSet

    if plan is None:
        plan = PLAN
    flat = [m for pr in plan for m in pr]
    assert sum(flat) == M_TOT
    n_tiles = len(flat)
    n_pairs = len(plan)
    m_count = Counter(flat)
    m2_count = Counter(sum(pr) for pr in plan)

    nc = bacc.Bacc(
        "TRN2", target_bir_lowering=False, debug=False, num_devices=N_CORES
    )
    x = nc.dram_tensor("x", [P, 3, M_TOT], _DT, kind="ExternalInput")
    # acc columns: one count col per tile + one max col per pair
    acc_cols = n_tiles + n_pairs
    out = nc.dram_tensor("out", [P, acc_cols], _DT, kind="ExternalOutput")

    with TileContext(nc) as tc:
        with (
            tc.tile_pool(name="xin", bufs=xbufs) as xpool,
            tc.tile_pool(name="small", bufs=4) as spool,
            tc.tile_pool(name="accp", bufs=1) as accpool,
        ):
            eng_of = {"dve": nc.vector, "pool": nc.gpsimd}
            accs = accpool.tile([P, acc_cols], _DT)
            # zero acc so unused columns (merged-max mode) sum to 0 on host
            nc.vector.memset(accs, 0.0)
            merged_pairs = set()
            if merge_max_tail:
                lo = n_pairs - sqrt_acc_last - merge_max_tail
                merged_pairs = set(range(max(0, lo), n_pairs - sqrt_acc_last))
            merged_m2 = sum(sum(plan[pj]) for pj in merged_pairs)
            if merged_m2:
                dmerge = accpool.tile([P, merged_m2], _BF, tag="dmerge")
            else:
                dmerge = None
            dm_off = [0]
            dm_state = {"col": None, "deps": []}
            scr16 = accpool.tile([P, 2], _BF)  # dummy-sqrt operand only

            # dummy sqrt: preload the ACT table set containing Sqrt+Square
            nc.vector.memset(scr16[:, :1], 1.0)
            nc.scalar.activation(
                out=scr16[:, :1], in_=scr16[:, :1],
                func=mybir.ActivationFunctionType.Sqrt,
            )

            acc_writers = []
            wb_idx = accpool.tile([P, 1], mybir.dt.int32)
            nc.gpsimd.memset(wb_idx, 0)
            wb_sem = nc.alloc_semaphore("wb_dma")
            # scratch accum target: walrus rejects tensor_scalar without an
            # accum_out, so the elementwise max accumulates here (unused)
            dummy_acc = accpool.tile([P, 1], _DT)

            gi = [0]  # global tile index
            m_off = [0]
            sq_act_of = {}  # tile -> its ACT square handle (or None)
            last_max = [None]  # most recent max/tsmax handle (DVE)
            count_of = {}  # tile -> count handle
            tsmax_of = {}  # pair -> tsmax handle (sqrt_acc path)
            max_of = {}  # pair -> max handle (regular path)
            sqrt_of = {}  # pair -> sqrt handle

            def stage_a(ti, M, s2, s_off, pi=None, last_in_pair=True):
                sz = str(M)
                nb = min(xbufs, m_count[M])
                xt = xpool.tile([P, 3, M], _DT, tag="xt" + sz, bufs=nb)
                m0 = m_off[0]
                r = nc.sync.dma_start(out=xt, in_=x[:, :, m0 : m0 + M])
                NAME_MAP[r.ins.name] = ("dma", ti)
                m_off[0] += M
                x2 = spool.tile([P, 3, M], _BF, tag="x2" + sz,
                                bufs=min(4, m_count[M]))
                ndve = tail_dve_sq and ti >= n_tiles - tail_dve_sq
                npool = tail_pool and ti >= n_tiles - tail_pool
                engs = sq_eng
                if tail_eng and ti >= n_tiles - tail_eng_n:
                    engs = tail_eng
                if last2_eng and ti >= n_tiles - 2:
                    engs = last2_eng
                sq_act = None
                for k, ename in enumerate(engs):
                    if ndve:
                        ename = "dve"
                    elif npool and k > 0:
                        # tail: y,z squares on Pool to unload DVE's queue
                        ename = "pool"
                    if ename == "act":
                        r = nc.scalar.activation(
                            out=x2[:, k, :], in_=xt[:, k, :],
                            func=mybir.ActivationFunctionType.Square,
                        )
                        sq_act = r
                    else:
                        r = eng_of[ename].tensor_tensor(
                            out=x2[:, k, :], in0=xt[:, k, :], in1=xt[:, k, :],
                            op=mybir.AluOpType.mult,
                        )
                    NAME_MAP[r.ins.name] = ("sq_" + "xyz"[k], ti)
                sq_act_of[ti] = sq_act
                sl = s2[:, s_off : s_off + M]
                r = nc.vector.tensor_tensor(
                    out=sl, in0=x2[:, 0, :], in1=x2[:, 1, :],
                    op=mybir.AluOpType.add,
                )
                NAME_MAP[r.ins.name] = ("add1", ti)
                if (pin_max_tail and ti >= n_tiles - pin_max_tail
                        and last_max[0] is not None):
                    # keep earlier pairs' max ops AHEAD of the tail adds on
                    # DVE so they don't pollute the endgame queue
                    r.ins.set_nosync_dependencies(NameSet(
                        list(r.ins.nosync_dependency_names())
                        + [last_max[0].ins.name]
                    ))
                r = nc.vector.tensor_tensor(
                    out=sl, in0=sl, in1=x2[:, 2, :], op=mybir.AluOpType.add,
                )
                NAME_MAP[r.ins.name] = ("add2", ti)
                # count(s > 1) -> acc col ti (4x TS; independent of sqrt).
                # pair-counted pairs emit ONE count over the full s2 on the
                # second tile (fewer endgame DVE ops).
                paired_ct = (pair_count_from is not None and pi is not None
                             and pi >= pair_count_from)
                if paired_ct and not last_in_pair:
                    return
                cin = s2[:, : s_off + M] if paired_ct else sl
                csz = s_off + M if paired_ct else M
                cscr = spool.tile([P, csz], _BF, tag="c" + str(csz),
                                  bufs=min(2, max(m_count[M], 1)))
                r = nc.vector.tensor_scalar(
                    out=cscr, in0=cin, scalar1=1.0, scalar2=None,
                    op0=mybir.AluOpType.is_gt, op1=mybir.AluOpType.add,
                    accum_out=accs[:, ti : ti + 1],
                )
                NAME_MAP[r.ins.name] = ("count", ti)
                count_of[ti] = r
                acc_writers.append(r)

            def stage_b(pi, M2, s2, after=None):
                sz = str(M2)
                use_sqrt_acc = sqrt_acc_last and pi >= n_pairs - sqrt_acc_last
                if pi in merged_pairs:
                    # write d into the shared merge buffer; ONE max op over
                    # the whole buffer is emitted after the loop
                    d = dmerge[:, dm_off[0] : dm_off[0] + M2]
                    dm_off[0] += M2
                    r = nc.scalar.activation(
                        out=d, in_=s2,
                        func=mybir.ActivationFunctionType.Sqrt,
                    )
                    NAME_MAP[r.ins.name] = ("sqrt", pi)
                    sqrt_of[pi] = r
                    if after is not None:
                        r.ins.set_nosync_dependencies(NameSet(
                            list(r.ins.nosync_dependency_names())
                            + [after.ins.name]
                        ))
                    if dm_state["col"] is None:
                        dm_state["col"] = n_tiles + pi
                    return
                d = spool.tile([P, M2], _BF, tag="d" + sz,
                               bufs=min(4, m2_count[M2]))
                if use_sqrt_acc:
                    # m = max(s,1) on DVE (4x, right after add2 in-queue),
                    # then ACT sqrt-with-accum: acc += sum(sqrt(m)) =
                    # sum(max(d,1)). Ends on ACT -> no post-sqrt DVE hop.
                    m = spool.tile([P, M2], _BF, tag="m" + sz,
                                   bufs=min(2, m2_count[M2]))
                    r = nc.vector.tensor_scalar(
                        out=m, in0=s2, scalar1=1.0, scalar2=None,
                        op0=mybir.AluOpType.max, op1=mybir.AluOpType.add,
                        accum_out=dummy_acc,
                    )
                    NAME_MAP[r.ins.name] = ("tsmax", pi)
                    tsmax_of[pi] = r
                    r = nc.scalar.activation(
                        out=d, in_=m, func=mybir.ActivationFunctionType.Sqrt,
                        accum_out=accs[:, n_tiles + pi : n_tiles + pi + 1],
                    )
                    NAME_MAP[r.ins.name] = ("sqrt", pi)
                    sqrt_of[pi] = r
                    if after is not None:
                        r.ins.set_nosync_dependencies(NameSet(
                            list(r.ins.nosync_dependency_names())
                            + [after.ins.name]
                        ))
                    acc_writers.append(r)
                    return
                r = nc.scalar.activation(
                    out=d, in_=s2, func=mybir.ActivationFunctionType.Sqrt,
                )
                NAME_MAP[r.ins.name] = ("sqrt", pi)
                sqrt_of[pi] = r
                if after is not None:
                    # scheduler-only edge: keep this sqrt BEHIND the newest
                    # tile's ACT square so squares stay DMA-anchored
                    r.ins.set_nosync_dependencies(NameSet(
                        list(r.ins.nosync_dependency_names())
                        + [after.ins.name]
                    ))
                # sum(max(d,1)) -> acc col n_tiles+pi (4x TS)
                mscr = spool.tile([P, M2], _BF, tag="mx" + sz,
                                  bufs=min(2, m2_count[M2]))
                r = nc.vector.tensor_scalar(
                    out=mscr, in0=d, scalar1=1.0, scalar2=None,
                    op0=mybir.AluOpType.max, op1=mybir.AluOpType.add,
                    accum_out=accs[:, n_tiles + pi : n_tiles + pi + 1],
                )
                NAME_MAP[r.ins.name] = ("max", pi)
                max_of[pi] = r
                acc_writers.append(r)
                last_max[0] = r

            pending = deque()
            for pi, pr in enumerate(plan):
                M2 = sum(pr)
                s2 = spool.tile([P, M2], _BF, tag="s" + str(M2),
                                bufs=min(4, m2_count[M2]))
                s_off = 0
                for mi, M in enumerate(pr):
                    stage_a(gi[0], M, s2, s_off, pi=pi,
                            last_in_pair=(mi == len(pr) - 1))
                    s_off += M
                    gi[0] += 1
                pending.append((pi, M2, s2))
                last_pair = pi == n_pairs - 1
                eff_lag = lag
                if pi >= n_pairs - tail_edge:
                    eff_lag = lag + tail_lag_extra
                if last_flush and last_pair:
                    eff_lag = 0
                while len(pending) > eff_lag:
                    if pi >= n_pairs - tail_edge:
                        # tail: order sqrts after the newest ACT square
                        aft = None
                        for tj in range(gi[0] - 1, -1, -1):
                            if sq_act_of.get(tj) is not None:
                                aft = sq_act_of[tj]
                                break
                    else:
                        aft = sq_act_of.get(gi[0] - len(pr))
                    stage_b(*pending.popleft(), after=aft)
            while pending:
                stage_b(*pending.popleft())

            if dmerge is not None and dm_state["col"] is not None:
                mscr = accpool.tile([P, merged_m2], _BF, tag="mscr")
                r = nc.vector.tensor_scalar(
                    out=mscr, in0=dmerge, scalar1=1.0, scalar2=None,
                    op0=mybir.AluOpType.max, op1=mybir.AluOpType.add,
                    accum_out=accs[:, dm_state["col"] : dm_state["col"] + 1],
                )
                NAME_MAP[r.ins.name] = ("maxM", 0)
                acc_writers.append(r)

            if endgame_edges:
                def add_nosync(ins_r, after_r):
                    nm = after_r.ins.name
                    if (nm in ins_r.ins.sync_dependency_names()
                            or nm in ins_r.ins.nosync_dependency_names()):
                        return
                    ins_r.ins.set_nosync_dependencies(NameSet(
                        list(ins_r.ins.nosync_dependency_names()) + [nm]
                    ))
                # keep late mid-pair maxes out of the endgame DVE chain:
                # order them after the last tile's count
                lc = count_of.get(n_tiles - 1)
                if lc is not None:
                    for pj in range(max(0, n_pairs - 4), n_pairs):
                        if pj in max_of:
                            add_nosync(max_of[pj], lc)
                # let the last counts overlap the final sqrt: order them
                # after the last pair's tsmax on DVE
                tm = tsmax_of.get(n_pairs - 1)
                if tm is not None:
                    for tj in range(max(0, n_tiles - eg_ct), n_tiles):
                        if tj in count_of:
                            add_nosync(count_of[tj], tm)
                # keep the tail sqrts behind the LAST ACT square so the
                # final tile's square is never queued behind them
                last_sq = None
                for tj in range(n_tiles - 1, -1, -1):
                    if sq_act_of.get(tj) is not None:
                        last_sq = sq_act_of[tj]
                        break
                if last_sq is not None:
                    for pj in range(max(0, n_pairs - 1 - eg_sq), n_pairs):
                        if pj in sqrt_of:
                            add_nosync(sqrt_of[pj], last_sq)

            # out-DMA via pre-staged SWDGE descriptors (prep early, trigger
            # after the final accumulate; RAW edges moved to the trigger)
            in_view = accs[:, :].rearrange("p (a b w) -> p a b w", a=1, b=1)
            out_view = out[:, :].rearrange("p (a b w) -> a p b w", a=1, b=1)
            wb_prep = nc.gpsimd.kv_writeback(
                out_view, in_view, wb_idx[:, :], prepare_only=True, sem=wb_sem,
            )
            wb_trig = nc.gpsimd.trigger_dma(count=None)
            acc_names = {w.ins.name for w in acc_writers}
            prep_sync = list(wb_prep.ins.sync_dependency_names())
            wb_prep.ins.set_sync_dependencies(
                NameSet([d for d in prep_sync if d not in acc_names])
            )
            wb_prep.ins.set_nosync_dependencies(NameSet(
                [d for d in wb_prep.ins.nosync_dependency_names()
                 if d not in acc_names]
            ))
            wb_trig.ins.set_sync_dependencies(NameSet(
                list(wb_trig.ins.sync_dependency_names()) + sorted(acc_names)
            ))

    nc.compile()

    # point the prep's on_update[0] at the DMASW drain sem (scatter_add-style
    # wiring; see v1 kernel for rationale)
    dmasw = None
    for i in nc.all_instructions():
        if i.sync_info:
            for w in i.sync_info.on_wait:
                if w.ant_name and w.ant_name.startswith("DMASW"):
                    dmasw = (w.id, w.ant_name)
    assert dmasw is not None, "no DMASW drain wait found"
    wb_prep.ins.sync_info.on_update[0] = mybir.SyncUpdate(
        sync_type="semaphore", id=dmasw[0], ant_name=dmasw[1],
        update_mode="sem-add-imm", update_value=16,
    )
    return nc


NAME_MAP = {}  # ins name -> (kind, index) for trace attribution

_nc_cache = None
last_results = None


def kernel(kps_world_pred: np.ndarray) -> np.ndarray:
    global _nc_cache, last_results
    x = np.ascontiguousarray(kps_world_pred, dtype=np.float32)
    assert x.shape == (B, J, D)

    # shard + deinterleave: [8, P, 8704 triplets, 3] -> [8, P, 3, 8704]
    v = np.ascontiguousarray(
        x.reshape(N_CORES, P, M_TOT, 3).transpose(0, 1, 3, 2)
    )
    in_maps = [{"x": v[c]} for c in range(N_CORES)]

    if _nc_cache is None:
        _nc_cache = build_nc()

    import time

    res = None
    for attempt in range(3):
        try:
            res = run_bass_kernel_spmd(_nc_cache, in_maps, list(range(N_CORES)))
            break
        except Exception:
            if attempt == 2:
                raise
            time.sleep(15)
    last_results = res

    # identity: sum(max(d,1)) + count(s>1) = masked_sum + P*M_TOT per core
    total = np.float64(0.0)
    for c in range(N_CORES):
        total += res.results[c]["out"].astype(np.float64).sum()
    total -= np.float64(N_CORES * P * M_TOT)
    return np.asarray(total / (B * J), dtype=np.float32)
